# revision 19
# baseline (speedup 1.0000x reference)
# Trainium2 Bass kernel for relative-position causal attention
# (Transformer-XL style: logits = q·k + q·table[n-m], causal softmax, AV, out-proj).
#
# Sharding: tensor-parallel over heads — 16 heads / 8 cores = 2 heads per core.
# Each core computes its heads' projections, attention, and a partial output
# projection [B,D,N] (fp16); the host sums the 8 partials in fp32.
#
# Precision: every logit-affecting matmul (q/k/table projections, content and
# position logits) runs as a 3-term fp16 hi/lo decomposition
#   A@B ~= Ah@Bh + Ah@Bl + Al@Bh   (hi = fp16(x), lo = fp16(x - hi))
# accumulated in fp32 PSUM. That carries ~22 mantissa bits through the PE at
# 1 cycle/col (vs 4 for true fp32) — host-side simulation shows the same
# softmax argmax decisions as the fp32 reference (end-to-end l2 ~7e-4).
# The value path (v proj, softmax weights, AV, out proj) is plain fp16.
#
# Position logits are computed in *diagonal* layout (T[ni, jr] = q[nb+ni] ·
# table[jr], a plain matmul since the table index is the diagonal n-m), then
# converted to row layout with a DMA "shear" through a DRAM scratch strip:
# partition ni reads flat offset 127 + ni*(W-1) + j, which is exactly the
# per-partition-shifted gather no on-chip engine can do.

from contextlib import ExitStack

import numpy as np

N = 2048
M = 2048
B = 2
D = 1024
H = 16
DQK = 64
DV = 64
NCORES = 8
HPC = H // NCORES  # heads per core = 2
NZ = N * B
KT = D // 128  # 8 contraction tiles
TW = 2048  # table width (diagonals 0..2047)

_cache = {}


def _sincos_rev():
    """sincos basis for diagonals d=0..2047, column-reversed, transposed to
    [D, TW] so sctr[:, jr] = sincos(d=2047-jr).  Computed with jax on CPU to
    match the reference's fp32 rounding of inv_freq/phases/sin bitwise."""
    try:
        import jax
        import jax.numpy as jnp

        cpu = jax.devices("cpu")[0]
        with jax.default_device(cpu):
            r = jnp.arange(0.0, float(TW), dtype=jnp.float32)
            inv_freq = 1.0 / (
                10000.0 ** (jnp.arange(0.0, D, 2.0, dtype=jnp.float32) / D)
            )
            phases = r[:, None] * inv_freq[None, :]
            sincos = jnp.concatenate([jnp.sin(phases), jnp.cos(phases)], axis=-1)
            sc = np.asarray(sincos)  # [TW, D]
    except Exception:
        r = np.arange(0.0, float(TW), dtype=np.float32)
        inv_freq = (
            1.0
            / (10000.0 ** (np.arange(0.0, D, 2.0, dtype=np.float32) / np.float32(D)))
        ).astype(np.float32)
        phases = (r[:, None] * inv_freq[None, :]).astype(np.float32)
        sc = np.concatenate(
            [np.sin(phases, dtype=np.float32), np.cos(phases, dtype=np.float32)],
            axis=-1,
        )
    return np.ascontiguousarray(sc[::-1].T.astype(np.float32))  # [D, TW]


def _build(loop=1, mode="full"):
    import concourse.bacc as bacc
    import concourse.mybir as mybir
    import concourse.tile as tile
    from concourse.bass import AP
    from concourse.masks import make_identity
    from concourse.tile_rust import add_dep_helper

    f32 = mybir.dt.float32
    f16 = mybir.dt.float16
    AX = mybir.AxisListType.X
    ADD = mybir.AluOpType.add
    MAX = mybir.AluOpType.max
    EXP = mybir.ActivationFunctionType.Exp

    nc = bacc.Bacc("TRN2", target_bir_lowering=False, debug=False, num_devices=NCORES)

    # fp16 hi/lo input pairs (host-split)
    xq_h = nc.dram_tensor("xq_h", [D, NZ], f16, kind="ExternalInput")
    xq_l = nc.dram_tensor("xq_l", [D, NZ], f16, kind="ExternalInput")
    xkv_h = nc.dram_tensor("xkv_h", [D, NZ], f16, kind="ExternalInput")
    xkv_l = nc.dram_tensor("xkv_l", [D, NZ], f16, kind="ExternalInput")
    sc_h = nc.dram_tensor("sc_h", [D, TW], f16, kind="ExternalInput")
    sc_l = nc.dram_tensor("sc_l", [D, TW], f16, kind="ExternalInput")
    wq_h = nc.dram_tensor("wq_h", [D, 128], f16, kind="ExternalInput")
    wq_l = nc.dram_tensor("wq_l", [D, 128], f16, kind="ExternalInput")
    wk_h = nc.dram_tensor("wk_h", [D, 128], f16, kind="ExternalInput")
    wk_l = nc.dram_tensor("wk_l", [D, 128], f16, kind="ExternalInput")
    wp_h = nc.dram_tensor("wp_h", [D, 128], f16, kind="ExternalInput")
    wp_l = nc.dram_tensor("wp_l", [D, 128], f16, kind="ExternalInput")
    wvT = nc.dram_tensor("wvT", [D, 128], f16, kind="ExternalInput")
    woT = nc.dram_tensor("woT", [128, D], f16, kind="ExternalInput")
    outT = nc.dram_tensor("outT", [B, D, N], f16, kind="ExternalOutput")

    # DRAM scratch strips for the diagonal->row shear, one per (z, h, i).
    scr = {}
    for z in range(B):
        for h in range(HPC):
            for i in range(16):
                W = 128 * (i + 1)
                scr[(z, h, i)] = nc.dram_tensor(
                    f"scr_{z}_{h}_{i}", [128 * W], f32, kind="Internal"
                )

    def r3(t):  # [D, C] dram -> [128, KT, C] partition view
        return t.ap().rearrange("(t p) n -> p t n", p=128)

    with tile.TileContext(nc) as tc:
        with ExitStack() as ctx:
            wpool = ctx.enter_context(tc.tile_pool(name="wpool", bufs=1))
            big = ctx.enter_context(tc.tile_pool(name="big", bufs=1))
            xp = ctx.enter_context(tc.tile_pool(name="xp", bufs=2))
            work = ctx.enter_context(tc.tile_pool(name="work", bufs=3))
            lpool = ctx.enter_context(tc.tile_pool(name="lpool", bufs=3))
            tpool = ctx.enter_context(tc.tile_pool(name="tpool", bufs=2))
            # PSUM: 2 + 2 + 3 + 1 = 8 banks
            paC = ctx.enter_context(tc.tile_pool(name="paC", bufs=2, space="PSUM"))
            paT = ctx.enter_context(tc.tile_pool(name="paT", bufs=2, space="PSUM"))
            paP = ctx.enter_context(tc.tile_pool(name="paP", bufs=3, space="PSUM"))
            paA = ctx.enter_context(tc.tile_pool(name="paA", bufs=1, space="PSUM"))
            ptsp = ctx.enter_context(tc.tile_pool(name="ptsp", bufs=4))

            # ---- constants / weights ----
            ident16 = wpool.tile([128, 128], f16)
            make_identity(nc, ident16[:])
            wq_sb = wpool.tile([128, KT, 2, 128], f16)  # [., t, hi/lo, .]
            wk_sb = wpool.tile([128, KT, 2, 128], f16)
            wp_sb = wpool.tile([128, KT, 2, 128], f16)
            wv_sb = wpool.tile([128, KT, 128], f16)
            wo_sb = wpool.tile([128, D], f16)
            nc.sync.dma_start(wq_sb[:, :, 0, :], r3(wq_h))
            nc.sync.dma_start(wq_sb[:, :, 1, :], r3(wq_l))
            nc.sync.dma_start(wk_sb[:, :, 0, :], r3(wk_h))
            nc.sync.dma_start(wk_sb[:, :, 1, :], r3(wk_l))
            nc.sync.dma_start(wp_sb[:, :, 0, :], r3(wp_h))
            nc.sync.dma_start(wp_sb[:, :, 1, :], r3(wp_l))
            nc.sync.dma_start(wv_sb[:], r3(wvT))
            nc.sync.dma_start(wo_sb[:], woT.ap())

            # ---- persistent activations ----
            q_sb = big.tile([128, 2, NZ], f16)  # [hd, hi/lo, z*N+n]
            k_sb = big.tile([128, 2, NZ], f16)
            tab_sb = big.tile([128, 2, TW], f16)  # reversed diag table
            V_all = big.tile([128, B * 16, 128], f16)  # [m-part, z*16+mb, hv]
            attn16 = big.tile([128, NZ], f16)  # [hv, z*N+n], normalized

            def mm3(ps, w_sb_c, x_h, x_l, first, last):
                """3-term hi/lo accumulation into psum ps."""
                nc.tensor.matmul(ps, w_sb_c[0], x_h, start=first, stop=False)
                nc.tensor.matmul(ps, w_sb_c[0], x_l, start=False, stop=False)
                nc.tensor.matmul(ps, w_sb_c[1], x_h, start=False, stop=last)

            def project(x_hd, x_ld, w_sb, out_sb, corder, do_v=None):
                """out_sb[:, 0/1, c-slice] = hi/lo of (w.T @ x); c-outer, t-inner.
                If do_v, also compute v = wv.T @ x_hi for the same chunks and
                transpose into V_all."""
                for c in corder:
                    x_t = xp.tile([128, KT, 512], f16, tag="xh")
                    x_tl = xp.tile([128, KT, 512], f16, tag="xl")
                    nc.sync.dma_start(x_t[:], x_hd[:, :, 512 * c : 512 * (c + 1)])
                    nc.sync.dma_start(x_tl[:], x_ld[:, :, 512 * c : 512 * (c + 1)])
                    ps = paC.tile([128, 512], f32, tag="cps", name=f"pj{c}")
                    for t in range(KT):
                        mm3(
                            ps[:],
                            (w_sb[:, t, 0, :], w_sb[:, t, 1, :]),
                            x_t[:, t, :],
                            x_tl[:, t, :],
                            t == 0,
                            t == KT - 1,
                        )
                    cs = slice(512 * c, 512 * (c + 1))
                    nc.scalar.copy(out_sb[:, 0, cs], ps[:])
                    nc.vector.tensor_sub(out_sb[:, 1, cs], ps[:], out_sb[:, 0, cs])
                    if do_v is not None:
                        vps = paT.tile([128, 512], f32, tag="tps")
                        for t in range(KT):
                            nc.tensor.matmul(
                                vps[:],
                                wv_sb[:, t, :],
                                x_t[:, t, :],
                                start=(t == 0),
                                stop=(t == KT - 1),
                            )
                        vT16 = work.tile([128, 512], f16, tag="vT")
                        nc.scalar.copy(vT16[:], vps[:])
                        z, mb0 = c // 4, (c % 4) * 4
                        for j in range(4):
                            vtp = paP.tile([128, 128], f16, tag="ptp")
                            nc.tensor.transpose(
                                vtp[:], vT16[:, 128 * j : 128 * (j + 1)], ident16[:]
                            )
                            nc.vector.tensor_copy(
                                V_all[:, z * 16 + mb0 + j, :], vtp[:]
                            )

            def phase_proj():
                # table first (descending chunks: attn strip i needs the LAST
                # W/512 chunks), then q, then k+v.
                project(r3(sc_h), r3(sc_l), wp_sb, tab_sb, [3, 2, 1, 0])
                project(r3(xq_h), r3(xq_l), wq_sb, q_sb, range(8))
                project(r3(xkv_h), r3(xkv_l), wk_sb, k_sb, range(8), do_v=True)

            def emit_av(z, h, i, P_all):
                """transpose P + AV + attn writeback for a finished strip.
                The AV matmul for tile mt is emitted two transposes behind, so
                the PSUM->SBUF copy of pt(mt) overlaps PE work instead of
                stalling the in-order PE queue on a DVE semaphore every tile."""
                hs = slice(64 * h, 64 * (h + 1))
                nb = 128 * i
                at_ps = paA.tile([64, 128], f32, tag="avp")
                DEPTH = 2
                pts = []

                def av(j):
                    nc.tensor.matmul(
                        at_ps[:],
                        V_all[:, z * 16 + j, hs],
                        pts[j][:],
                        start=(j == 0),
                        stop=(j == i),
                    )

                for mt in range(i + 1):
                    ptp = paP.tile([128, 128], f16, tag="ptp")
                    nc.tensor.transpose(
                        ptp[:], P_all[:, 128 * mt : 128 * (mt + 1)], ident16[:]
                    )
                    pt_sb = ptsp.tile([128, 128], f16, tag="pts")
                    nc.vector.tensor_copy(pt_sb[:], ptp[:])
                    pts.append(pt_sb)
                    if mt >= DEPTH:
                        av(mt - DEPTH)
                for j in range(max(0, i + 1 - DEPTH), i + 1):
                    av(j)
                nc.scalar.copy(attn16[hs, z * N + nb : z * N + nb + 128], at_ps[:])

            w_insts = {}

            def emit_pos_strip(z, h, i):
                """Diagonal-layout position product for one strip, written to
                its DRAM shear strip. The write is issued from the ACT ring
                (nc.scalar): its wait (the ACT tT copies just ahead of it on
                the same queue) is trivially satisfied, so it never head-of-
                line-blocks the shear reads on the sync ring."""
                hs = slice(64 * h, 64 * (h + 1))
                nb = 128 * i
                W = nb + 128
                nbc = (W + 511) // 512
                q_hi = q_sb[hs, 0, z * N + nb : z * N + nb + 128]
                q_lo = q_sb[hs, 1, z * N + nb : z * N + nb + 128]
                tT_sb = tpool.tile([128, 2048], f32, tag="tT")
                for c in range(nbc):
                    wc = min(512, W - 512 * c)
                    t_ps = paT.tile([128, 512], f32, tag="tps")
                    tc0 = TW - W + 512 * c
                    nc.tensor.matmul(
                        t_ps[:, :wc], q_hi, tab_sb[hs, 0, tc0 : tc0 + wc],
                        start=True, stop=False,
                    )
                    nc.tensor.matmul(
                        t_ps[:, :wc], q_hi, tab_sb[hs, 1, tc0 : tc0 + wc],
                        start=False, stop=False,
                    )
                    nc.tensor.matmul(
                        t_ps[:, :wc], q_lo, tab_sb[hs, 0, tc0 : tc0 + wc],
                        start=False, stop=True,
                    )
                    nc.scalar.copy(tT_sb[:, 512 * c : 512 * c + wc], t_ps[:, :wc])
                w_insts[(z, h, i)] = nc.scalar.dma_start(
                    AP(scr[(z, h, i)], 0, [[W, 128], [1, W]]), tT_sb[:, :W]
                )

            def phase_attn(z, h, next_group=None):
                """Software-pipelined: AV of strip i-1 is emitted after the
                logits/softmax of strip i so the PE always has independent
                matmul work while strip i's shear read is in flight."""
                hs = slice(64 * h, 64 * (h + 1))
                prev = None
                for i in range(16):
                    # interleave the NEXT group's position/shear strip here so
                    # its PE/ACT/DMA work overlaps this group's DVE-heavy
                    # softmax chain (and its writes complete long before read)
                    if next_group is not None:
                        emit_pos_strip(next_group[0], next_group[1], i)
                    nb = 128 * i
                    W = nb + 128
                    nbc = (W + 511) // 512
                    q_hi = q_sb[hs, 0, z * N + nb : z * N + nb + 128]
                    q_lo = q_sb[hs, 1, z * N + nb : z * N + nb + 128]

                    L_sb = lpool.tile([128, 2048], f32, tag="L")
                    Lp = lpool.tile([128, 2048], f32, tag="Lp")
                    cmax = work.tile([128, 4], f32, tag="cmax")
                    r_inst = nc.sync.dma_start(
                        Lp[:, :W], AP(scr[(z, h, i)], 127, [[W - 1, 128], [1, W]])
                    )
                    add_dep_helper(
                        r_inst.ins,
                        w_insts[(z, h, i)].ins,
                        reason="shear read after strip write",
                    )
                    # causal triangle mask on the last 128 cols (j - nb > p)
                    nc.gpsimd.affine_select(
                        Lp[:, nb : nb + 128],
                        Lp[:, nb : nb + 128],
                        compare_op=mybir.AluOpType.is_ge,
                        fill=-1e30,
                        base=0,
                        pattern=[[-1, 128]],
                        channel_multiplier=1,
                    )

                    # --- content logits (emitted after the read: psum tiles are
                    # released quickly by the ttr, and the PE fills the shear
                    # latency with the previous strip's AV work) ---
                    for c in range(nbc):
                        wc = min(512, W - 512 * c)
                        c_ps = paC.tile([128, 512], f32, tag="cps")
                        kc0 = z * N + 512 * c
                        nc.tensor.matmul(
                            c_ps[:, :wc], q_hi, k_sb[hs, 0, kc0 : kc0 + wc],
                            start=True, stop=False,
                        )
                        nc.tensor.matmul(
                            c_ps[:, :wc], q_hi, k_sb[hs, 1, kc0 : kc0 + wc],
                            start=False, stop=False,
                        )
                        nc.tensor.matmul(
                            c_ps[:, :wc], q_lo, k_sb[hs, 0, kc0 : kc0 + wc],
                            start=False, stop=True,
                        )
                        # NOTE: tensor_tensor_reduce would fuse these two, but
                        # it hard-crashes the exec unit on TRN2 HW (fine in sim).
                        nc.vector.tensor_add(
                            L_sb[:, 512 * c : 512 * c + wc],
                            c_ps[:, :wc],
                            Lp[:, 512 * c : 512 * c + wc],
                        )
                        nc.vector.reduce_max(
                            out=cmax[:, c : c + 1],
                            in_=L_sb[:, 512 * c : 512 * c + wc],
                            axis=AX,
                        )
                    negmax = work.tile([128, 1], f32, tag="negmax")
                    nc.vector.tensor_reduce(
                        out=negmax[:], in_=cmax[:, :nbc], axis=AX, op=MAX, negate=True
                    )

                    if prev is not None:
                        emit_softmax_av(z, h, *prev)
                    prev = (i, L_sb, negmax)
                emit_softmax_av(z, h, *prev)

            def emit_softmax_av(z, h, i, L_sb, negmax):
                # wide exp (+total sumexp), reciprocal, wide normalize, then AV.
                # Emitted one strip behind the logits so the ACT/DVE softmax
                # chain of strip i overlaps the PE/DVE logit work of strip i+1.
                W = 128 * (i + 1)
                ssum = work.tile([128, 1], f32, tag="ssum")
                P_all = lpool.tile([128, 2048], f16, tag="Pall")
                nc.scalar.activation(
                    P_all[:, :W], L_sb[:, :W], EXP,
                    bias=negmax[:], scale=1.0, accum_out=ssum[:],
                )
                rsum = work.tile([128, 1], f32, tag="rsum")
                nc.vector.reciprocal(rsum[:], ssum[:])
                nc.vector.tensor_scalar_mul(P_all[:, :W], P_all[:, :W], rsum[:])
                emit_av(z, h, i, P_all)

            def phase_out(z):
                for dc in range(8):
                    for nn in range(4):
                        o_ps = paC.tile([128, 512], f32, tag="cps")
                        nc.tensor.matmul(
                            o_ps[:],
                            wo_sb[:, 128 * dc : 128 * (dc + 1)],
                            attn16[:, z * N + 512 * nn : z * N + 512 * (nn + 1)],
                            start=True,
                            stop=True,
                        )
                        o_sb = work.tile([128, 512], f16, tag="osb")
                        nc.scalar.copy(o_sb[:], o_ps[:])
                        nc.sync.dma_start(
                            outT.ap()[
                                z, 128 * dc : 128 * (dc + 1), 512 * nn : 512 * (nn + 1)
                            ],
                            o_sb[:],
                        )

            def loop_body():
                phase_proj()
                if mode == "proj":
                    # debug: dump projections via outT (fp16), skip attention
                    o0 = outT.ap()[0].rearrange("d n -> (d n)").rearrange(
                        "(p a) -> p a", p=128
                    )
                    o1 = outT.ap()[1].rearrange("d n -> (d n)").rearrange(
                        "(p a) -> p a", p=128
                    )
                    nc.sync.dma_start(o0[:, :8192], q_sb[:].rearrange("p a b -> p (a b)"))
                    nc.sync.dma_start(o1[:, :8192], k_sb[:].rearrange("p a b -> p (a b)"))
                    return
                groups = [(z, h) for z in range(B) for h in range(HPC)]
                for i in range(16):
                    emit_pos_strip(*groups[0], i)
                for gi, (z, h) in enumerate(groups):
                    nxt = groups[gi + 1] if gi + 1 < len(groups) else None
                    phase_attn(z, h, next_group=nxt)
                    if h == HPC - 1:
                        phase_out(z)

            if loop == 1:
                loop_body()
            else:
                with tc.For_i(0, loop, 1):
                    loop_body()

    nc.compile()
    return nc


def _hl(a):
    hi = a.astype(np.float16)
    lo = (a - hi.astype(np.float32)).astype(np.float16)
    return np.ascontiguousarray(hi), np.ascontiguousarray(lo)


def _prep_inputs(x_q, x_kv, to_q, to_kv, for_pos_enc, to_o):
    xqT = np.asarray(x_q, dtype=np.float32).transpose(2, 1, 0).reshape(D, NZ)
    xkvT = np.asarray(x_kv, dtype=np.float32).transpose(2, 1, 0).reshape(D, NZ)
    xq_h, xq_l = _hl(xqT)
    xkv_h, xkv_l = _hl(xkvT)
    if "sc" not in _cache:
        _cache["sc"] = _hl(_sincos_rev())
    sc_h, sc_l = _cache["sc"]
    to_q = np.asarray(to_q, dtype=np.float32)
    to_kv = np.asarray(to_kv, dtype=np.float32)
    fpe = np.asarray(for_pos_enc, dtype=np.float32)
    to_o = np.asarray(to_o, dtype=np.float32)
    in_maps = []
    for c in range(NCORES):
        hs = slice(HPC * c, HPC * (c + 1))
        wq_hi, wq_lo = _hl(to_q[hs].reshape(HPC * DQK, D).T.copy())
        wk_hi, wk_lo = _hl(to_kv[hs, :DQK].reshape(HPC * DQK, D).T.copy())
        wp_hi, wp_lo = _hl(fpe[hs].reshape(HPC * DQK, D).T.copy())
        wv = np.ascontiguousarray(
            to_kv[hs, DQK:].reshape(HPC * DV, D).T.astype(np.float16)
        )
        wo = np.ascontiguousarray(
            to_o[:, hs, :].reshape(D, HPC * DV).T.astype(np.float16)
        )
        in_maps.append(
            {
                "xq_h": xq_h,
                "xq_l": xq_l,
                "xkv_h": xkv_h,
                "xkv_l": xkv_l,
                "sc_h": sc_h,
                "sc_l": sc_l,
                "wq_h": wq_hi,
                "wq_l": wq_lo,
                "wk_h": wk_hi,
                "wk_l": wk_lo,
                "wp_h": wp_hi,
                "wp_l": wp_lo,
                "wvT": wv,
                "woT": wo,
            }
        )
    return in_maps


def kernel(x_q, x_kv, to_q, to_kv, for_pos_enc, to_o):
    from concourse.bass_utils import run_bass_kernel_spmd

    if "nc" not in _cache:
        _cache["nc"] = _build()
    nc = _cache["nc"]
    in_maps = _prep_inputs(x_q, x_kv, to_q, to_kv, for_pos_enc, to_o)
    res = run_bass_kernel_spmd(nc, in_maps, core_ids=list(range(NCORES)))
    acc = np.zeros((B, D, N), dtype=np.float32)
    for c in range(NCORES):
        acc += res.results[c]["outT"].astype(np.float32)
    return np.ascontiguousarray(acc.transpose(2, 0, 1)).astype(np.float32)


# revision 20
# speedup vs baseline: 1.1993x; 1.1993x over previous
# Trainium2 Bass kernel for relative-position causal attention
# (Transformer-XL style: logits = q·k + q·table[n-m], causal softmax, AV, out-proj).
#
# Sharding: tensor-parallel over heads — 16 heads / 8 cores = 2 heads per core.
# Each core computes its heads' projections, attention, and a partial output
# projection [B,D,N] (fp16); the host sums the 8 partials in fp32.
#
# Precision: every logit-affecting matmul (q/k/table projections, content and
# position logits) runs as a 3-term fp16 hi/lo decomposition
#   A@B ~= Ah@Bh + Ah@Bl + Al@Bh   (hi = fp16(x), lo = fp16(x - hi))
# accumulated in fp32 PSUM. That carries ~22 mantissa bits through the PE at
# 1 cycle/col (vs 4 for true fp32) — host-side simulation shows the same
# softmax argmax decisions as the fp32 reference (end-to-end l2 ~7e-4).
# The value path (v proj, softmax weights, AV, out proj) is plain fp16.
#
# Position logits are computed in *diagonal* layout (T[ni, jr] = q[nb+ni] ·
# table[jr], a plain matmul since the table index is the diagonal n-m), then
# converted to row layout with a DMA "shear" through a DRAM scratch strip:
# partition ni reads flat offset 127 + ni*(W-1) + j, which is exactly the
# per-partition-shifted gather no on-chip engine can do.

from contextlib import ExitStack

import numpy as np

N = 2048
M = 2048
B = 2
D = 1024
H = 16
DQK = 64
DV = 64
NCORES = 8
HPC = H // NCORES  # heads per core = 2
NZ = N * B
KT = D // 128  # 8 contraction tiles
TW = 2048  # table width (diagonals 0..2047)

_cache = {}


def _sincos_rev():
    """sincos basis for diagonals d=0..2047, column-reversed, transposed to
    [D, TW] so sctr[:, jr] = sincos(d=2047-jr).  Computed with jax on CPU to
    match the reference's fp32 rounding of inv_freq/phases/sin bitwise."""
    try:
        import jax
        import jax.numpy as jnp

        cpu = jax.devices("cpu")[0]
        with jax.default_device(cpu):
            r = jnp.arange(0.0, float(TW), dtype=jnp.float32)
            inv_freq = 1.0 / (
                10000.0 ** (jnp.arange(0.0, D, 2.0, dtype=jnp.float32) / D)
            )
            phases = r[:, None] * inv_freq[None, :]
            sincos = jnp.concatenate([jnp.sin(phases), jnp.cos(phases)], axis=-1)
            sc = np.asarray(sincos)  # [TW, D]
    except Exception:
        r = np.arange(0.0, float(TW), dtype=np.float32)
        inv_freq = (
            1.0
            / (10000.0 ** (np.arange(0.0, D, 2.0, dtype=np.float32) / np.float32(D)))
        ).astype(np.float32)
        phases = (r[:, None] * inv_freq[None, :]).astype(np.float32)
        sc = np.concatenate(
            [np.sin(phases, dtype=np.float32), np.cos(phases, dtype=np.float32)],
            axis=-1,
        )
    return np.ascontiguousarray(sc[::-1].T.astype(np.float32))  # [D, TW]


def _build(loop=1, mode="full"):
    import concourse.bacc as bacc
    import concourse.mybir as mybir
    import concourse.tile as tile
    from concourse.bass import AP
    from concourse.masks import make_identity
    from concourse.tile_rust import add_dep_helper

    f32 = mybir.dt.float32
    f16 = mybir.dt.float16
    AX = mybir.AxisListType.X
    ADD = mybir.AluOpType.add
    MAX = mybir.AluOpType.max
    EXP = mybir.ActivationFunctionType.Exp

    nc = bacc.Bacc("TRN2", target_bir_lowering=False, debug=False, num_devices=NCORES)

    # fp16 hi/lo input pairs (host-split)
    xq_h = nc.dram_tensor("xq_h", [D, NZ], f16, kind="ExternalInput")
    xq_l = nc.dram_tensor("xq_l", [D, NZ], f16, kind="ExternalInput")
    xkv_h = nc.dram_tensor("xkv_h", [D, NZ], f16, kind="ExternalInput")
    xkv_l = nc.dram_tensor("xkv_l", [D, NZ], f16, kind="ExternalInput")
    sc_h = nc.dram_tensor("sc_h", [D, TW], f16, kind="ExternalInput")
    sc_l = nc.dram_tensor("sc_l", [D, TW], f16, kind="ExternalInput")
    wq_h = nc.dram_tensor("wq_h", [D, 128], f16, kind="ExternalInput")
    wq_l = nc.dram_tensor("wq_l", [D, 128], f16, kind="ExternalInput")
    wk_h = nc.dram_tensor("wk_h", [D, 128], f16, kind="ExternalInput")
    wk_l = nc.dram_tensor("wk_l", [D, 128], f16, kind="ExternalInput")
    wp_h = nc.dram_tensor("wp_h", [D, 128], f16, kind="ExternalInput")
    wp_l = nc.dram_tensor("wp_l", [D, 128], f16, kind="ExternalInput")
    wvT = nc.dram_tensor("wvT", [D, 128], f16, kind="ExternalInput")
    woT = nc.dram_tensor("woT", [128, D], f16, kind="ExternalInput")
    outT = nc.dram_tensor("outT", [B, D, N], f16, kind="ExternalOutput")

    # DRAM scratch strips for the diagonal->row shear, one per (z, h, i).
    scr = {}
    for z in range(B):
        for h in range(HPC):
            for i in range(16):
                W = 128 * (i + 1)
                scr[(z, h, i)] = nc.dram_tensor(
                    f"scr_{z}_{h}_{i}", [128 * W], f32, kind="Internal"
                )

    def r3(t):  # [D, C] dram -> [128, KT, C] partition view
        return t.ap().rearrange("(t p) n -> p t n", p=128)

    with tile.TileContext(nc) as tc:
        with ExitStack() as ctx:
            wpool = ctx.enter_context(tc.tile_pool(name="wpool", bufs=1))
            big = ctx.enter_context(tc.tile_pool(name="big", bufs=1))
            xp = ctx.enter_context(tc.tile_pool(name="xp", bufs=2))
            work = ctx.enter_context(tc.tile_pool(name="work", bufs=3))
            lpool = ctx.enter_context(tc.tile_pool(name="lpool", bufs=3))
            tpool = ctx.enter_context(tc.tile_pool(name="tpool", bufs=2))
            # PSUM: 2 + 2 + 3 + 1 = 8 banks
            paC = ctx.enter_context(tc.tile_pool(name="paC", bufs=2, space="PSUM"))
            paT = ctx.enter_context(tc.tile_pool(name="paT", bufs=2, space="PSUM"))
            paP = ctx.enter_context(tc.tile_pool(name="paP", bufs=3, space="PSUM"))
            paA = ctx.enter_context(tc.tile_pool(name="paA", bufs=1, space="PSUM"))
            ptsp = ctx.enter_context(tc.tile_pool(name="ptsp", bufs=4))

            # ---- constants / weights ----
            ident16 = wpool.tile([128, 128], f16)
            make_identity(nc, ident16[:])
            wq_sb = wpool.tile([128, KT, 2, 128], f16)  # [., t, hi/lo, .]
            wk_sb = wpool.tile([128, KT, 2, 128], f16)
            wp_sb = wpool.tile([128, KT, 2, 128], f16)
            wv_sb = wpool.tile([128, KT, 128], f16)
            wo_sb = wpool.tile([128, D], f16)
            nc.sync.dma_start(wq_sb[:, :, 0, :], r3(wq_h))
            nc.sync.dma_start(wq_sb[:, :, 1, :], r3(wq_l))
            nc.sync.dma_start(wk_sb[:, :, 0, :], r3(wk_h))
            nc.sync.dma_start(wk_sb[:, :, 1, :], r3(wk_l))
            nc.sync.dma_start(wp_sb[:, :, 0, :], r3(wp_h))
            nc.sync.dma_start(wp_sb[:, :, 1, :], r3(wp_l))
            nc.sync.dma_start(wv_sb[:], r3(wvT))
            nc.sync.dma_start(wo_sb[:], woT.ap())

            # ---- persistent activations ----
            q_sb = big.tile([128, 2, NZ], f16)  # [hd, hi/lo, z*N+n]
            k_sb = big.tile([128, 2, NZ], f16)
            tab_sb = big.tile([128, 2, TW], f16)  # reversed diag table
            V_all = big.tile([128, B * 16, 128], f16)  # [m-part, z*16+mb, hv]
            attn16 = big.tile([128, NZ], f16)  # [hv, z*N+n], normalized

            def mm3(ps, w_sb_c, x_h, x_l, first, last):
                """3-term hi/lo accumulation into psum ps."""
                nc.tensor.matmul(ps, w_sb_c[0], x_h, start=first, stop=False)
                nc.tensor.matmul(ps, w_sb_c[0], x_l, start=False, stop=False)
                nc.tensor.matmul(ps, w_sb_c[1], x_h, start=False, stop=last)

            def project(x_hd, x_ld, w_sb, out_sb, corder, do_v=None):
                """out_sb[:, 0/1, c-slice] = hi/lo of (w.T @ x); c-outer, t-inner.
                If do_v, also compute v = wv.T @ x_hi for the same chunks and
                transpose into V_all."""
                for c in corder:
                    x_t = xp.tile([128, KT, 512], f16, tag="xh")
                    x_tl = xp.tile([128, KT, 512], f16, tag="xl")
                    nc.sync.dma_start(x_t[:], x_hd[:, :, 512 * c : 512 * (c + 1)])
                    nc.sync.dma_start(x_tl[:], x_ld[:, :, 512 * c : 512 * (c + 1)])
                    ps = paC.tile([128, 512], f32, tag="cps", name=f"pj{c}")
                    for t in range(KT):
                        mm3(
                            ps[:],
                            (w_sb[:, t, 0, :], w_sb[:, t, 1, :]),
                            x_t[:, t, :],
                            x_tl[:, t, :],
                            t == 0,
                            t == KT - 1,
                        )
                    cs = slice(512 * c, 512 * (c + 1))
                    nc.scalar.copy(out_sb[:, 0, cs], ps[:])
                    nc.vector.tensor_sub(out_sb[:, 1, cs], ps[:], out_sb[:, 0, cs])
                    if do_v is not None:
                        vps = paT.tile([128, 512], f32, tag="tps")
                        for t in range(KT):
                            nc.tensor.matmul(
                                vps[:],
                                wv_sb[:, t, :],
                                x_t[:, t, :],
                                start=(t == 0),
                                stop=(t == KT - 1),
                            )
                        vT16 = work.tile([128, 512], f16, tag="vT")
                        nc.scalar.copy(vT16[:], vps[:])
                        z, mb0 = c // 4, (c % 4) * 4
                        for j in range(4):
                            vtp = paP.tile([128, 128], f16, tag="ptp")
                            nc.tensor.transpose(
                                vtp[:], vT16[:, 128 * j : 128 * (j + 1)], ident16[:]
                            )
                            nc.vector.tensor_copy(
                                V_all[:, z * 16 + mb0 + j, :], vtp[:]
                            )

            def phase_proj():
                # table first (descending chunks: attn strip i needs the LAST
                # W/512 chunks), then q, then k+v.
                project(r3(sc_h), r3(sc_l), wp_sb, tab_sb, [3, 2, 1, 0])
                project(r3(xq_h), r3(xq_l), wq_sb, q_sb, range(8))
                project(r3(xkv_h), r3(xkv_l), wk_sb, k_sb, range(8), do_v=True)

            def emit_av(z, h, i, P_all):
                """transpose P + AV + attn writeback for a finished strip.
                The AV matmul for tile mt is emitted two transposes behind, so
                the PSUM->SBUF copy of pt(mt) overlaps PE work instead of
                stalling the in-order PE queue on a DVE semaphore every tile."""
                hs = slice(64 * h, 64 * (h + 1))
                nb = 128 * i
                at_ps = paA.tile([64, 128], f32, tag="avp")
                DEPTH = 2
                pts = []

                def av(j):
                    nc.tensor.matmul(
                        at_ps[:],
                        V_all[:, z * 16 + j, hs],
                        pts[j][:],
                        start=(j == 0),
                        stop=(j == i),
                    )

                for mt in range(i + 1):
                    ptp = paP.tile([128, 128], f16, tag="ptp")
                    nc.tensor.transpose(
                        ptp[:], P_all[:, 128 * mt : 128 * (mt + 1)], ident16[:]
                    )
                    pt_sb = ptsp.tile([128, 128], f16, tag="pts")
                    nc.vector.tensor_copy(pt_sb[:], ptp[:])
                    pts.append(pt_sb)
                    if mt >= DEPTH:
                        av(mt - DEPTH)
                for j in range(max(0, i + 1 - DEPTH), i + 1):
                    av(j)
                nc.scalar.copy(attn16[hs, z * N + nb : z * N + nb + 128], at_ps[:])

            w_insts = {}

            def phase_pos(z, h):
                """Diagonal-layout position products for ALL strips of (z,h),
                written to their DRAM shear strips up-front — by the time the
                attention loop reads a strip, its write has long completed, so
                the DMA round-trip latency never sits on the critical path."""
                hs = slice(64 * h, 64 * (h + 1))
                for i in range(16):
                    nb = 128 * i
                    W = nb + 128
                    nbc = (W + 511) // 512
                    q_hi = q_sb[hs, 0, z * N + nb : z * N + nb + 128]
                    q_lo = q_sb[hs, 1, z * N + nb : z * N + nb + 128]
                    tT_sb = tpool.tile([128, 2048], f32, tag="tT")
                    for c in range(nbc):
                        wc = min(512, W - 512 * c)
                        t_ps = paT.tile([128, 512], f32, tag="tps")
                        tc0 = TW - W + 512 * c
                        nc.tensor.matmul(
                            t_ps[:, :wc], q_hi, tab_sb[hs, 0, tc0 : tc0 + wc],
                            start=True, stop=False,
                        )
                        nc.tensor.matmul(
                            t_ps[:, :wc], q_hi, tab_sb[hs, 1, tc0 : tc0 + wc],
                            start=False, stop=False,
                        )
                        nc.tensor.matmul(
                            t_ps[:, :wc], q_lo, tab_sb[hs, 0, tc0 : tc0 + wc],
                            start=False, stop=True,
                        )
                        nc.scalar.copy(tT_sb[:, 512 * c : 512 * c + wc], t_ps[:, :wc])
                    w_insts[(z, h, i)] = nc.sync.dma_start(
                        AP(scr[(z, h, i)], 0, [[W, 128], [1, W]]), tT_sb[:, :W]
                    )

            def phase_attn(z, h):
                """Software-pipelined: AV of strip i-1 is emitted after the
                logits/softmax of strip i so the PE always has independent
                matmul work while strip i's shear read is in flight."""
                hs = slice(64 * h, 64 * (h + 1))
                prev = None
                for i in range(16):
                    nb = 128 * i
                    W = nb + 128
                    nbc = (W + 511) // 512
                    q_hi = q_sb[hs, 0, z * N + nb : z * N + nb + 128]
                    q_lo = q_sb[hs, 1, z * N + nb : z * N + nb + 128]

                    L_sb = lpool.tile([128, 2048], f32, tag="L")
                    Lp = lpool.tile([128, 2048], f32, tag="Lp")
                    cmax = work.tile([128, 4], f32, tag="cmax")
                    r_inst = nc.sync.dma_start(
                        Lp[:, :W], AP(scr[(z, h, i)], 127, [[W - 1, 128], [1, W]])
                    )
                    add_dep_helper(
                        r_inst.ins,
                        w_insts[(z, h, i)].ins,
                        reason="shear read after strip write",
                    )
                    # causal triangle mask on the last 128 cols (j - nb > p)
                    nc.gpsimd.affine_select(
                        Lp[:, nb : nb + 128],
                        Lp[:, nb : nb + 128],
                        compare_op=mybir.AluOpType.is_ge,
                        fill=-1e30,
                        base=0,
                        pattern=[[-1, 128]],
                        channel_multiplier=1,
                    )

                    # --- content logits (emitted after the read: psum tiles are
                    # released quickly by the ttr, and the PE fills the shear
                    # latency with the previous strip's AV work) ---
                    for c in range(nbc):
                        wc = min(512, W - 512 * c)
                        c_ps = paC.tile([128, 512], f32, tag="cps")
                        kc0 = z * N + 512 * c
                        nc.tensor.matmul(
                            c_ps[:, :wc], q_hi, k_sb[hs, 0, kc0 : kc0 + wc],
                            start=True, stop=False,
                        )
                        nc.tensor.matmul(
                            c_ps[:, :wc], q_hi, k_sb[hs, 1, kc0 : kc0 + wc],
                            start=False, stop=False,
                        )
                        nc.tensor.matmul(
                            c_ps[:, :wc], q_lo, k_sb[hs, 0, kc0 : kc0 + wc],
                            start=False, stop=True,
                        )
                        # NOTE: tensor_tensor_reduce would fuse these two, but
                        # it hard-crashes the exec unit on TRN2 HW (fine in sim).
                        nc.vector.tensor_add(
                            L_sb[:, 512 * c : 512 * c + wc],
                            c_ps[:, :wc],
                            Lp[:, 512 * c : 512 * c + wc],
                        )
                        nc.vector.reduce_max(
                            out=cmax[:, c : c + 1],
                            in_=L_sb[:, 512 * c : 512 * c + wc],
                            axis=AX,
                        )
                    negmax = work.tile([128, 1], f32, tag="negmax")
                    nc.vector.tensor_reduce(
                        out=negmax[:], in_=cmax[:, :nbc], axis=AX, op=MAX, negate=True
                    )

                    if prev is not None:
                        emit_softmax_av(z, h, *prev)
                    prev = (i, L_sb, negmax)
                emit_softmax_av(z, h, *prev)

            def emit_softmax_av(z, h, i, L_sb, negmax):
                # wide exp (+total sumexp), reciprocal, wide normalize, then AV.
                # Emitted one strip behind the logits so the ACT/DVE softmax
                # chain of strip i overlaps the PE/DVE logit work of strip i+1.
                W = 128 * (i + 1)
                ssum = work.tile([128, 1], f32, tag="ssum")
                P_all = lpool.tile([128, 2048], f16, tag="Pall")
                nc.scalar.activation(
                    P_all[:, :W], L_sb[:, :W], EXP,
                    bias=negmax[:], scale=1.0, accum_out=ssum[:],
                )
                rsum = work.tile([128, 1], f32, tag="rsum")
                nc.vector.reciprocal(rsum[:], ssum[:])
                nc.vector.tensor_scalar_mul(P_all[:, :W], P_all[:, :W], rsum[:])
                emit_av(z, h, i, P_all)

            def phase_out(z):
                for dc in range(8):
                    for nn in range(4):
                        o_ps = paC.tile([128, 512], f32, tag="cps")
                        nc.tensor.matmul(
                            o_ps[:],
                            wo_sb[:, 128 * dc : 128 * (dc + 1)],
                            attn16[:, z * N + 512 * nn : z * N + 512 * (nn + 1)],
                            start=True,
                            stop=True,
                        )
                        o_sb = work.tile([128, 512], f16, tag="osb")
                        nc.scalar.copy(o_sb[:], o_ps[:])
                        nc.sync.dma_start(
                            outT.ap()[
                                z, 128 * dc : 128 * (dc + 1), 512 * nn : 512 * (nn + 1)
                            ],
                            o_sb[:],
                        )

            def loop_body():
                phase_proj()
                if mode == "proj":
                    # debug: dump projections via outT (fp16), skip attention
                    o0 = outT.ap()[0].rearrange("d n -> (d n)").rearrange(
                        "(p a) -> p a", p=128
                    )
                    o1 = outT.ap()[1].rearrange("d n -> (d n)").rearrange(
                        "(p a) -> p a", p=128
                    )
                    nc.sync.dma_start(o0[:, :8192], q_sb[:].rearrange("p a b -> p (a b)"))
                    nc.sync.dma_start(o1[:, :8192], k_sb[:].rearrange("p a b -> p (a b)"))
                    return
                for z in range(B):
                    for h in range(HPC):
                        phase_pos(z, h)
                        phase_attn(z, h)
                    phase_out(z)

            if loop == 1:
                loop_body()
            else:
                with tc.For_i(0, loop, 1):
                    loop_body()

    nc.compile()
    return nc


def _hl(a):
    hi = a.astype(np.float16)
    lo = (a - hi.astype(np.float32)).astype(np.float16)
    return np.ascontiguousarray(hi), np.ascontiguousarray(lo)


def _prep_inputs(x_q, x_kv, to_q, to_kv, for_pos_enc, to_o):
    xqT = np.asarray(x_q, dtype=np.float32).transpose(2, 1, 0).reshape(D, NZ)
    xkvT = np.asarray(x_kv, dtype=np.float32).transpose(2, 1, 0).reshape(D, NZ)
    xq_h, xq_l = _hl(xqT)
    xkv_h, xkv_l = _hl(xkvT)
    if "sc" not in _cache:
        _cache["sc"] = _hl(_sincos_rev())
    sc_h, sc_l = _cache["sc"]
    to_q = np.asarray(to_q, dtype=np.float32)
    to_kv = np.asarray(to_kv, dtype=np.float32)
    fpe = np.asarray(for_pos_enc, dtype=np.float32)
    to_o = np.asarray(to_o, dtype=np.float32)
    in_maps = []
    for c in range(NCORES):
        hs = slice(HPC * c, HPC * (c + 1))
        wq_hi, wq_lo = _hl(to_q[hs].reshape(HPC * DQK, D).T.copy())
        wk_hi, wk_lo = _hl(to_kv[hs, :DQK].reshape(HPC * DQK, D).T.copy())
        wp_hi, wp_lo = _hl(fpe[hs].reshape(HPC * DQK, D).T.copy())
        wv = np.ascontiguousarray(
            to_kv[hs, DQK:].reshape(HPC * DV, D).T.astype(np.float16)
        )
        wo = np.ascontiguousarray(
            to_o[:, hs, :].reshape(D, HPC * DV).T.astype(np.float16)
        )
        in_maps.append(
            {
                "xq_h": xq_h,
                "xq_l": xq_l,
                "xkv_h": xkv_h,
                "xkv_l": xkv_l,
                "sc_h": sc_h,
                "sc_l": sc_l,
                "wq_h": wq_hi,
                "wq_l": wq_lo,
                "wk_h": wk_hi,
                "wk_l": wk_lo,
                "wp_h": wp_hi,
                "wp_l": wp_lo,
                "wvT": wv,
                "woT": wo,
            }
        )
    return in_maps


def kernel(x_q, x_kv, to_q, to_kv, for_pos_enc, to_o):
    from concourse.bass_utils import run_bass_kernel_spmd

    if "nc" not in _cache:
        _cache["nc"] = _build()
    nc = _cache["nc"]
    in_maps = _prep_inputs(x_q, x_kv, to_q, to_kv, for_pos_enc, to_o)
    res = run_bass_kernel_spmd(nc, in_maps, core_ids=list(range(NCORES)))
    acc = np.zeros((B, D, N), dtype=np.float32)
    for c in range(NCORES):
        acc += res.results[c]["outT"].astype(np.float32)
    return np.ascontiguousarray(acc.transpose(2, 0, 1)).astype(np.float32)
